# revision 1
# baseline (speedup 1.0000x reference)
"""Trainium2 Bass kernel for nn_BasicTransformerBlock_12738873000028.

Strategy (8 NeuronCores): data-parallel over batch (2) x sequence-parallel over
query rows (4) => core c handles batch c//4, query rows [(c%4)*1024, +1024).

Everything on-chip runs in "transposed" layout (channels on SBUF partitions,
tokens on the free dim), so every matmul contracts over the partition dim with
zero on-chip transposes. Host-side numpy does the layout transposes, bf16
casts, bias-row augmentation and sharding; matmuls are bf16 with fp32 PSUM
accumulation, everything else (softmax, norm stats, residuals) is fp32.

Softmax denominator comes from a ones-column appended to V (one extra PSUM
row); biases ride as an extra contraction row (ones row in the activations,
bias row in the weights). The group-norm statistics are the only cross-core
dependency: a 16x2 fp32 AllReduce within each batch's 4-core group.
"""

import numpy as np
import ml_dtypes

import concourse.bacc as bacc
import concourse.tile as tile
from concourse import mybir
from concourse.bass_utils import run_bass_kernel_spmd

bf16 = ml_dtypes.bfloat16
F32 = mybir.dt.float32
BF16 = mybir.dt.bfloat16

B, H, W, C = 2, 64, 64, 320
N = H * W                      # 4096 tokens per batch
NCORES = 8
QL = N // 4                    # 1024 local query rows per core
QWIN = 512                     # query window (fp32 PSUM bank = 512 floats)
NQW = QL // QWIN               # 2 windows
C8 = 8 * C                     # 2560
C4 = 4 * C                     # 1280
GROUPS, EPS = 16, 1e-3
GSIZE = C // GROUPS            # 20 channels per group
GCNT = float(N * GSIZE)        # elements per (batch, group)
MT = N // 128                  # 32 key tiles
HT8 = C8 // 128                # 20 geglu output tiles
HT4 = C4 // 128                # 10 per half
SCALE = float(C) ** -0.5

# channel tiling: (offset, size, augmented-size)
CT = [(0, 128, 128), (128, 128, 128), (256, 64, 65)]

_NC_CACHE = {}


def _emit_body(nc, tc, ap, pools, upto=99, accum_out=False):
    """Emit one full forward pass. ap: dict of DRAM APs. pools: tile pools."""
    res, ps_acc, ps_mm, ps_tiny, dram = (
        pools["res"], pools["acc"], pools["mm"], pools["tiny"], pools["dram"])

    def rtile(shape, dtype, tag):
        return res.tile(shape, dtype, tag=tag, name=tag)

    # ---------------- resident loads ----------------
    X16 = []   # x^T bf16 (+ones row), full batch [321, 4096]
    XQ16 = []  # x^T bf16 local query cols (+ones row) [321, 1024]
    XF = []    # x^T fp32 local [320, 1024]
    GB = []    # gamma/beta [320, 2]
    INDsb = []
    for i, (off, sz, asz) in enumerate(CT):
        t = rtile([asz, N], BF16, f"x16_{i}")
        X16.append(t)
        t = rtile([asz, QL], BF16, f"xq16_{i}")
        nc.sync.dma_start(out=t, in_=ap["xq16"][off:off + asz, :])
        XQ16.append(t)
        t = rtile([sz, QL], F32, f"xf_{i}")
        nc.sync.dma_start(out=t, in_=ap["xt32"][off:off + sz, :])
        XF.append(t)
        t = rtile([sz, 2], F32, f"gb_{i}")
        nc.sync.dma_start(out=t, in_=ap["gb"][off:off + sz, :])
        GB.append(t)
        t = rtile([sz, GROUPS], F32, f"ind_{i}")
        nc.sync.dma_start(out=t, in_=ap["ind"][off:off + sz, :])
        INDsb.append(t)
    for i, (off, sz, asz) in enumerate(CT):
        nc.sync.dma_start(out=X16[i], in_=ap["xt16"][off:off + asz, :])
    INDT = rtile([GROUPS, C], F32, "indt")
    nc.sync.dma_start(out=INDT, in_=ap["indt"][:, :])

    def load_w(name, ncols):
        tiles = []
        for i, (off, sz, asz) in enumerate(CT):
            t = rtile([asz, ncols], BF16, f"w_{name}_{i}")
            nc.sync.dma_start(out=t, in_=ap[name][off:off + asz, :])
            tiles.append(t)
        return tiles

    SAQ = load_w("saq", C)
    SAK = load_w("sak", C)
    SAV = load_w("sav", C)
    SAP = load_w("sap", C)
    CAQ = load_w("caq", C)
    CAK = load_w("cak", C)
    CAV = load_w("cav", C)
    CAP = load_w("cap", C)
    GW = load_w("gw", C8)
    DW = []
    for j in range(HT4):
        t = rtile([128, C], BF16, f"dw_{j}")
        nc.sync.dma_start(out=t, in_=ap["dw"][j * 128:(j + 1) * 128, :])
        DW.append(t)
    DB = rtile([1, C], BF16, "db")
    nc.sync.dma_start(out=DB, in_=ap["db"][:, :])

    # persistent on-chip state
    K16 = [rtile([sz, N], BF16, f"k16_{i}") for i, (_, sz, _) in enumerate(CT)]
    V16 = rtile([128, MT * 321], BF16, "v16")   # per key-tile: 320 cols V + 1 col ones
    Q16 = [rtile([sz, QL], BF16, f"q16_{i}") for i, (_, sz, _) in enumerate(CT)]
    X2 = [rtile([sz, QL], F32, f"x2_{i}") for i, (_, sz, _) in enumerate(CT)]
    X3 = [rtile([sz, QL], F32, f"x3_{i}") for i, (_, sz, _) in enumerate(CT)]
    XN16 = [rtile([asz, QL], BF16, f"xn16_{i}") for i, (_, _, asz) in enumerate(CT)]
    X316 = [rtile([asz, QL], BF16, f"x316_{i}") for i, (_, _, asz) in enumerate(CT)]
    YT = [rtile([sz, QL], F32, f"xf_{i}") for i, (_, sz, _) in enumerate(CT)]  # reuse xf slots
    ONES32 = rtile([1, 128], F32, "ones32")
    ONES16 = rtile([1, QWIN], BF16, "ones16")

    nc.vector.memset(ONES32, 1.0)
    nc.vector.memset(ONES16, 1.0)
    # ones column of every V key-tile block
    v_as_blocks = V16.rearrange("p (t c) -> p t c", c=321)
    nc.vector.memset(v_as_blocks[:, :, 320:321], 1.0)
    nc.vector.memset(XN16[2][64:65, :], 1.0)
    nc.vector.memset(X316[2][64:65, :], 1.0)

    def kv_proj(SRC16, WK, WV):
        """K^T[c, m] for all m into K16; V[m, c] (+ones col) into V16."""
        for mw in range(N // QWIN):
            for i, (off, sz, _) in enumerate(CT):
                pk = ps_mm.tile([sz, QWIN], F32, tag="mm", name="mm")
                for ci in range(3):
                    nc.tensor.matmul(
                        pk, WK[ci][:, off:off + sz],
                        SRC16[ci][:, mw * QWIN:(mw + 1) * QWIN],
                        start=(ci == 0), stop=(ci == 2))
                nc.scalar.copy(out=K16[i][:, mw * QWIN:(mw + 1) * QWIN], in_=pk)
        for mt in range(MT):
            pv = ps_mm.tile([128, C], F32, tag="mm", name="mm")
            for ci in range(3):
                nc.tensor.matmul(
                    pv, SRC16[ci][:, mt * 128:(mt + 1) * 128], WV[ci][:, :],
                    start=(ci == 0), stop=(ci == 2))
            nc.vector.tensor_copy(out=V16[:, mt * 321:mt * 321 + C], in_=pv)

    def q_proj(SRC16, WQ):
        for i, (off, sz, _) in enumerate(CT):
            for qw in range(NQW):
                pq = ps_mm.tile([sz, QWIN], F32, tag="mm", name="mm")
                for ci in range(3):
                    nc.tensor.matmul(
                        pq, WQ[ci][:, off:off + sz],
                        SRC16[ci][:, qw * QWIN:(qw + 1) * QWIN],
                        start=(ci == 0), stop=(ci == 2))
                nc.scalar.copy(out=Q16[i][:, qw * QWIN:(qw + 1) * QWIN], in_=pq)

    def attention_core(WP, resid_fn):
        """scores -> softmax -> SV -> div -> proj; resid_fn(co, qw, psum_p)."""
        for qw in range(NQW):
            qsl = slice(qw * QWIN, (qw + 1) * QWIN)
            po = [ps_acc.tile([asz, QWIN], F32, tag="acc", name="acc") for (_, _, asz) in CT]
            for mt in range(MT):
                psc = ps_mm.tile([128, QWIN], F32, tag="mm", name="mm")
                for ci in range(3):
                    nc.tensor.matmul(
                        psc, K16[ci][:, mt * 128:(mt + 1) * 128], Q16[ci][:, qsl],
                        start=(ci == 0), stop=(ci == 2))
                es = res.tile([128, QWIN], BF16, tag="es", name="es", bufs=3)
                nc.scalar.activation(out=es, in_=psc, func=mybir.ActivationFunctionType.Exp)
                for cj, (off, sz, asz) in enumerate(CT):
                    nc.tensor.matmul(
                        po[cj], V16[:, mt * 321 + off:mt * 321 + off + asz], es,
                        start=(mt == 0), stop=(mt == MT - 1))
            # softmax denominator: row 64 of po[2] is sum(exp)
            rec = res.tile([1, QWIN], F32, tag="rec", name="rec", bufs=1)
            nc.vector.reciprocal(rec, po[2][64:65, :])
            pb = ps_mm.tile([128, QWIN], F32, tag="mm", name="mm")
            nc.tensor.matmul(pb, ONES32, rec, start=True, stop=True)
            dbc = res.tile([128, QWIN], F32, tag="dbc", name="dbc", bufs=1)
            nc.scalar.copy(out=dbc, in_=pb)
            at = []
            for cj, (off, sz, asz) in enumerate(CT):
                t = res.tile([asz, QWIN], BF16, tag=f"at_{cj}", name=f"at_{cj}", bufs=2)
                nc.vector.tensor_mul(t, po[cj], dbc[0:asz, :])
                at.append(t)
            for co, (off, sz, _) in enumerate(CT):
                pp = ps_mm.tile([sz, QWIN], F32, tag="mm", name="mm")
                for cj in range(3):
                    nc.tensor.matmul(
                        pp, WP[cj][:, off:off + sz], at[cj],
                        start=(cj == 0), stop=(cj == 2))
                resid_fn(co, qw, pp)

    # ======== attn1 (self-attention) ========
    q_proj(XQ16, SAQ)
    kv_proj(X16, SAK, SAV)
    if upto <= 1:
        return

    def resid1(co, qw, pp):
        qsl = slice(qw * QWIN, (qw + 1) * QWIN)
        # x2 = 2*x + attn1
        nc.vector.scalar_tensor_tensor(
            out=X2[co][:, qsl], in0=XF[co][:, qsl], scalar=2.0, in1=pp,
            op0=mybir.AluOpType.mult, op1=mybir.AluOpType.add)

    attention_core(SAP, resid1)
    if upto <= 2:
        return

    # ======== group-norm stats + AllReduce ========
    s12 = [res.tile([sz, 2], F32, tag=f"s12_{i}", name=f"s12_{i}", bufs=1) for i, (_, sz, _) in enumerate(CT)]
    scratch = res.tile([128, QL], F32, tag="scratch", name="scratch", bufs=1)
    for i, (_, sz, _) in enumerate(CT):
        nc.vector.reduce_sum(out=s12[i][:, 0:1], in_=X2[i], axis=mybir.AxisListType.X)
        nc.scalar.activation(
            out=scratch[0:sz, :], in_=X2[i],
            func=mybir.ActivationFunctionType.Square, accum_out=s12[i][:, 1:2])
    pg = ps_tiny.tile([GROUPS, 2], F32, tag="tiny", name="tiny")
    for i in range(3):
        nc.tensor.matmul(pg, INDsb[i], s12[i], start=(i == 0), stop=(i == 2))
    g12 = res.tile([GROUPS, 2], F32, tag="g12", name="g12", bufs=1)
    nc.vector.tensor_copy(out=g12, in_=pg)
    ccin = dram.tile([GROUPS, 2], F32, tag="ccin", name="ccin")
    ccout = dram.tile([GROUPS, 2], F32, tag="ccout", name="ccout")
    nc.sync.dma_start(out=ccin, in_=g12)
    if not globals().get("_SKIP_COLLECTIVE"):
        nc.gpsimd.collective_compute(
            "AllReduce", mybir.AluOpType.add,
            replica_groups=[[0, 1, 2, 3], [4, 5, 6, 7]],
            ins=[ccin.opt()], outs=[ccout.opt()])
    else:
        nc.sync.dma_start(out=ccout, in_=ccin)
    gg = res.tile([GROUPS, 2], F32, tag="gg", name="gg", bufs=1)
    nc.sync.dma_start(out=gg, in_=ccout)

    # ======== attn2 K/V from context (independent of stats -> overlaps) ====
    C16 = []
    for i, (off, sz, asz) in enumerate(CT):
        t = rtile([asz, N], BF16, f"x16_{i}")  # reuse x16 slots
        C16.append(t)
    for i, (off, sz, asz) in enumerate(CT):
        nc.sync.dma_start(out=C16[i], in_=ap["ct16"][off:off + asz, :])
    kv_proj(C16, CAK, CAV)
    if upto <= 3:
        return

    # ======== finish group norm ========
    gtmp = res.tile([GROUPS, 4], F32, tag="gtmp", name="gtmp", bufs=1)
    grp2 = res.tile([GROUPS, 2], F32, tag="grp2", name="grp2", bufs=1)
    inv = 1.0 / GCNT
    nc.vector.tensor_scalar_mul(out=grp2[:, 1:2], in0=gg[:, 0:1], scalar1=inv)   # mean
    nc.vector.tensor_scalar_mul(out=gtmp[:, 0:1], in0=gg[:, 1:2], scalar1=inv)   # E[x^2]
    nc.vector.tensor_mul(gtmp[:, 1:2], grp2[:, 1:2], grp2[:, 1:2])               # mean^2
    nc.vector.tensor_sub(gtmp[:, 2:3], gtmp[:, 0:1], gtmp[:, 1:2])               # var
    epst = res.tile([GROUPS, 1], F32, tag="epst", name="epst", bufs=1)
    nc.vector.memset(epst, float(EPS))
    nc.scalar.activation(out=gtmp[:, 3:4], in_=gtmp[:, 2:3],
                         func=mybir.ActivationFunctionType.Sqrt, bias=epst)
    nc.vector.reciprocal(grp2[:, 0:1], gtmp[:, 3:4])                             # rstd
    for i, (off, sz, _) in enumerate(CT):
        pc = ps_tiny.tile([sz, 2], F32, tag="tiny", name="tiny")
        nc.tensor.matmul(pc, INDT[:, off:off + sz], grp2, start=True, stop=True)
        scs = res.tile([sz, 4], F32, tag=f"scs_{i}", name=f"scs_{i}", bufs=1)
        nc.vector.tensor_mul(scs[:, 0:1], pc[:, 0:1], GB[i][:, 0:1])     # scale=rstd*gamma
        nc.vector.tensor_mul(scs[:, 3:4], pc[:, 1:2], scs[:, 0:1])      # mean*scale
        nc.vector.tensor_sub(scs[:, 1:2], GB[i][:, 1:2], scs[:, 3:4])   # shift
        nc.vector.tensor_scalar_add(out=scs[:, 2:3], in0=scs[:, 0:1], scalar1=1.0)
        # xn (bf16, for Q2 projection)
        nc.vector.tensor_scalar(
            out=XN16[i][0:sz, :], in0=X2[i], scalar1=scs[:, 0:1], scalar2=scs[:, 1:2],
            op0=mybir.AluOpType.mult, op1=mybir.AluOpType.add)
        # x2 <- x2 + xn  (= x2*(1+scale) + shift), fp32, in place
        nc.vector.tensor_scalar(
            out=X2[i], in0=X2[i], scalar1=scs[:, 2:3], scalar2=scs[:, 1:2],
            op0=mybir.AluOpType.mult, op1=mybir.AluOpType.add)

    # ======== attn2 ========
    q_proj(XN16, CAQ)

    def resid2(co, qw, pp):
        qsl = slice(qw * QWIN, (qw + 1) * QWIN)
        # x3 = (x2 + xn) + attn2
        nc.vector.tensor_add(X3[co][:, qsl], X2[co][:, qsl], pp)

    attention_core(CAP, resid2)
    for i, (_, sz, _) in enumerate(CT):
        nc.vector.tensor_copy(out=X316[i][0:sz, :], in_=X3[i])
    if upto <= 4:
        return

    # ======== GEGLU FFN ========
    for qw in range(NQW):
        qsl = slice(qw * QWIN, (qw + 1) * QWIN)
        py = [ps_acc.tile([sz, QWIN], F32, tag="acc", name="acc")
              for (_, sz, _) in CT]
        for hh in range(HT4):
            pa = ps_mm.tile([128, QWIN], F32, tag="mm", name="mm")
            pgg = ps_mm.tile([128, QWIN], F32, tag="mm", name="mm")
            for ci in range(3):
                nc.tensor.matmul(
                    pa, GW[ci][:, hh * 128:(hh + 1) * 128], X316[ci][:, qsl],
                    start=(ci == 0), stop=(ci == 2))
            for ci in range(3):
                nc.tensor.matmul(
                    pgg, GW[ci][:, C4 + hh * 128:C4 + (hh + 1) * 128], X316[ci][:, qsl],
                    start=(ci == 0), stop=(ci == 2))
            sg = res.tile([128, QWIN], F32, tag="sg", name="sg", bufs=2)
            nc.scalar.activation(out=sg, in_=pgg,
                                 func=mybir.ActivationFunctionType.Sigmoid, scale=1.702)
            gsg = res.tile([128, QWIN], BF16, tag="gsg", name="gsg", bufs=2)
            nc.vector.tensor_mul(gsg, pgg, sg)
            t = res.tile([128, QWIN], BF16, tag="ff", name="ff", bufs=3)
            nc.vector.tensor_mul(t, pa, gsg)
            for co, (off, sz, _) in enumerate(CT):
                nc.tensor.matmul(py[co], DW[hh][:, off:off + sz], t,
                                 start=(hh == 0), stop=False)
        for co, (off, sz, _) in enumerate(CT):
            nc.tensor.matmul(py[co], DB[:, off:off + sz], ONES16,
                             start=False, stop=True)
            nc.vector.tensor_add(YT[co][:, qsl], py[co], X3[co][:, qsl])

    for i, (off, sz, _) in enumerate(CT):
        if accum_out:
            nc.gpsimd.dma_start(out=ap["yt"][off:off + sz, :], in_=YT[i],
                                accum_op=mybir.AluOpType.add)
        else:
            nc.sync.dma_start(out=ap["yt"][off:off + sz, :], in_=YT[i])
    if "tick" in ap:
        tick = res.tile([1, 4], F32, tag="tick", name="tick", bufs=1)
        for i in range(3):
            nc.vector.tensor_copy(out=tick[0:1, i:i + 1],
                                  in_=YT[i][0:1, QL - 1:QL])
        nc.sync.dma_start(out=ap["tick"], in_=tick)


def _build(rep=1, accum_out=False, tick=False):
    key = (rep, accum_out, tick)
    if key in _NC_CACHE:
        return _NC_CACHE[key]
    nc = bacc.Bacc("TRN2", target_bir_lowering=False, debug=False, num_devices=NCORES)
    shapes = {
        "xt16": ([C + 1, N], BF16), "xq16": ([C + 1, QL], BF16),
        "ct16": ([C + 1, N], BF16), "xt32": ([C, QL], F32),
        "saq": ([C + 1, C], BF16), "sak": ([C + 1, C], BF16),
        "sav": ([C + 1, C], BF16), "sap": ([C + 1, C], BF16),
        "caq": ([C + 1, C], BF16), "cak": ([C + 1, C], BF16),
        "cav": ([C + 1, C], BF16), "cap": ([C + 1, C], BF16),
        "gw": ([C + 1, C8], BF16), "dw": ([C4, C], BF16), "db": ([1, C], BF16),
        "gb": ([C, 2], F32), "ind": ([C, GROUPS], F32), "indt": ([GROUPS, C], F32),
    }
    ap = {}
    for name, (shape, dt) in shapes.items():
        ap[name] = nc.dram_tensor(name, shape, dt, kind="ExternalInput").ap()
    ap["yt"] = nc.dram_tensor("yt", [C, QL], F32, kind="ExternalOutput").ap()
    if tick:
        ap["tick"] = nc.dram_tensor("tick", [1, 4], F32, kind="ExternalOutput").ap()

    with tile.TileContext(nc) as tc:
        with (
            tc.tile_pool(name="res", bufs=1) as res,
            tc.tile_pool(name="acc", bufs=3, space="PSUM") as acc,
            tc.tile_pool(name="mm", bufs=4, space="PSUM") as mm,
            tc.tile_pool(name="tiny", bufs=1, space="PSUM") as tiny,
            tc.tile_pool(name="dram", bufs=1, space="DRAM") as dram,
        ):
            pools = {"res": res, "acc": acc, "mm": mm, "tiny": tiny, "dram": dram}
            for _ in range(rep):
                _emit_body(nc, tc, ap, pools, accum_out=accum_out)
    nc.finalize()
    _NC_CACHE[key] = nc
    return nc


def _prep_inputs(inputs):
    """Host-side sharding/layout prep. Returns in_maps for the 8 cores."""
    f32 = np.float32

    def aug(w, b, scale=1.0):
        w = np.asarray(w, f32) * scale
        b = np.asarray(b, f32).reshape(1, -1) * scale
        return np.ascontiguousarray(np.concatenate([w, b], axis=0)).astype(bf16)

    x = np.asarray(inputs["x"], f32).reshape(B, N, C)
    ctx = np.asarray(inputs["context"], f32).reshape(B, N, C)
    xt = np.ascontiguousarray(x.transpose(0, 2, 1))      # [B, C, N] fp32
    ctxt = np.ascontiguousarray(ctx.transpose(0, 2, 1))

    ones_row = np.ones((1, N), f32)
    xt16 = [np.concatenate([xt[b], ones_row], axis=0).astype(bf16) for b in range(B)]
    ct16 = [np.concatenate([ctxt[b], ones_row], axis=0).astype(bf16) for b in range(B)]

    weights = {
        "saq": aug(inputs["sa_q_w"], inputs["sa_q_b"], SCALE),
        "sak": aug(inputs["sa_k_w"], inputs["sa_k_b"]),
        "sav": aug(inputs["sa_v_w"], inputs["sa_v_b"]),
        "sap": aug(inputs["sa_p_w"], inputs["sa_p_b"]),
        "caq": aug(inputs["ca_q_w"], inputs["ca_q_b"], SCALE),
        "cak": aug(inputs["ca_k_w"], inputs["ca_k_b"]),
        "cav": aug(inputs["ca_v_w"], inputs["ca_v_b"]),
        "cap": aug(inputs["ca_p_w"], inputs["ca_p_b"]),
        "gw": aug(inputs["geglu_w"], inputs["geglu_b"]),
        "dw": np.asarray(inputs["dense_w"], f32).astype(bf16),
        "db": np.asarray(inputs["dense_b"], f32).reshape(1, C).astype(bf16),
    }
    gb = np.stack([np.asarray(inputs["ca_norm_g"], f32),
                   np.asarray(inputs["ca_norm_b"], f32)], axis=1)  # [C, 2]
    ind = np.zeros((C, GROUPS), f32)
    ind[np.arange(C), np.arange(C) // GSIZE] = 1.0
    indt = np.ascontiguousarray(ind.T)

    in_maps = []
    for c in range(NCORES):
        b = c // 4
        q0 = (c % 4) * QL
        m = {
            "xt16": xt16[b],
            "xq16": np.ascontiguousarray(xt16[b][:, q0:q0 + QL]),
            "ct16": ct16[b],
            "xt32": np.ascontiguousarray(xt[b][:, q0:q0 + QL]),
            "gb": gb, "ind": ind, "indt": indt,
        }
        m.update(weights)
        in_maps.append(m)
    return in_maps


def kernel(**inputs):
    in_maps = _prep_inputs(inputs)
    nc = _build()
    res = run_bass_kernel_spmd(nc, in_maps, list(range(NCORES)))
    out = np.zeros((B, N, C), np.float32)
    for c in range(NCORES):
        b = c // 4
        q0 = (c % 4) * QL
        out[b, q0:q0 + QL, :] = res.results[c]["yt"].T
    return out.reshape(B, H, W, C)


def _build_single(rep=1, upto=99):
    """Single-core, collective-free variant for TimelineSim analysis."""
    import concourse.bacc as _bacc
    nc = _bacc.Bacc("TRN2", target_bir_lowering=False, debug=False, num_devices=1)
    shapes = {
        "xt16": ([C + 1, N], BF16), "xq16": ([C + 1, QL], BF16),
        "ct16": ([C + 1, N], BF16), "xt32": ([C, QL], F32),
        "saq": ([C + 1, C], BF16), "sak": ([C + 1, C], BF16),
        "sav": ([C + 1, C], BF16), "sap": ([C + 1, C], BF16),
        "caq": ([C + 1, C], BF16), "cak": ([C + 1, C], BF16),
        "cav": ([C + 1, C], BF16), "cap": ([C + 1, C], BF16),
        "gw": ([C + 1, C8], BF16), "dw": ([C4, C], BF16), "db": ([1, C], BF16),
        "gb": ([C, 2], F32), "ind": ([C, GROUPS], F32), "indt": ([GROUPS, C], F32),
    }
    ap = {}
    for name, (shape, dt) in shapes.items():
        ap[name] = nc.dram_tensor(name, shape, dt, kind="ExternalInput").ap()
    ap["yt"] = nc.dram_tensor("yt", [C, QL], F32, kind="ExternalOutput").ap()
    globals()["_SKIP_COLLECTIVE"] = True
    try:
        with tile.TileContext(nc) as tc:
            with (
                tc.tile_pool(name="res", bufs=1) as res,
                tc.tile_pool(name="acc", bufs=3, space="PSUM") as acc,
                tc.tile_pool(name="mm", bufs=4, space="PSUM") as mm,
                tc.tile_pool(name="tiny", bufs=1, space="PSUM") as tiny,
                tc.tile_pool(name="dram", bufs=1, space="DRAM") as dram,
            ):
                pools = {"res": res, "acc": acc, "mm": mm, "tiny": tiny, "dram": dram}
                for _ in range(rep):
                    _emit_body(nc, tc, ap, pools, upto=upto)
    finally:
        globals()["_SKIP_COLLECTIVE"] = False
    nc.finalize()
    return nc



# revision 60
# speedup vs baseline: 5.5049x; 5.5049x over previous
"""Trainium2 Bass kernel for nn_BasicTransformerBlock_12738873000028.

Strategy (8 NeuronCores): data-parallel over batch (2) x sequence-parallel over
query rows (4) => core c handles batch c//4, query rows [(c%4)*1024, +1024).

v2: fp8 (e4m3) DoubleRow matmuls for every big GEMM, with two algebraic
rewrites that eliminate most on-chip data movement:

  * scores = x'_q (Wq Wk^T)' x'_m^T: the folded matrix M' (host-side, fp32)
    turns Q-proj+K-proj+QK into ONE small projection (QM = M'^T x'_q) plus
    score matmuls against the raw transposed input -- no K tensor on chip.
  * attn V-side reassociated: U = X^T Es (contract keys first), then
    pv = Wv'^T U -- no V tensor, no per-key-tile PSUM->SBUF copies.

All fp8 operands are pre-scaled (weights x32, M x256, U/at x64 via the
softmax denominator row) to sit in e4m3's normal range; descales are folded
into activation/copy scales and residual scalar_tensor_tensor ops.  Residual
path, group-norm stats and softmax accumulation stay fp32.  The group-norm
statistics are the only cross-core dependency: a 16x2 fp32 AllReduce within
each batch's 4-core group.

Layout conventions ("pair" tiles for DoubleRow, contraction on partitions):
  A-tile [128, 2, n]: subtile t holds rows 128t..128t+127 (channels 0..255)
  B-tile [128, 2, n]: subtile 0 holds rows 256.. (incl. bias/ones row),
                      subtile 1 is all zeros.
  key-pair tiles [128, 2, n]: subtile t holds key tile 2j+t.
"""

import struct

import numpy as np
import ml_dtypes

import concourse.bacc as bacc
import concourse.tile as tile
from concourse import mybir
from concourse.bass_utils import run_bass_kernel_spmd

f8 = ml_dtypes.float8_e4m3
F32 = mybir.dt.float32
BF16 = mybir.dt.bfloat16
F8 = mybir.dt.float8e4
DR = mybir.MatmulPerfMode.DoubleRow
AF = mybir.ActivationFunctionType
AL = mybir.AluOpType

B, H, W, C = 2, 64, 64, 320
N = H * W                      # 4096 tokens per batch
NCORES = 8
QL = N // 4                    # 1024 local query rows per core
QWIN = 512
NQW = QL // QWIN               # 2
MT = N // 128                  # 32 key tiles
NJ = MT // 2                   # 16 key-tile pairs
GROUPS, EPS = 16, 1e-3
GSIZE = C // GROUPS
GCNT = float(N * GSIZE)
SCALE = float(C) ** -0.5
WS = 32.0                      # weight fp8 pre-scale
MS = 256.0                     # folded-M fp8 pre-scale
US = 64.0                      # U / at fp8 pre-scale (via denom row)
C4, C8 = 4 * C, 8 * C
MPAD = 336                     # padded out-channel count for M' (321 -> 336)

OT = [(0, 128), (128, 128), (256, 65)]    # tiles incl extra (bias/denom) row
OTW = [(0, 128), (128, 128), (256, 64)]   # plain channel tiles

# f32 bit patterns whose 4 bytes are fp8/bf16 constants 1.0 / 64.0
F8_ONES_PAT = float(np.frombuffer(np.full(4, 1.0, f8).tobytes(), np.float32)[0])
F8_64_PAT = float(np.frombuffer(np.full(4, 64.0, f8).tobytes(), np.float32)[0])
BF16_ONES_PAT = float(np.frombuffer(
    np.full(2, 1.0, ml_dtypes.bfloat16).tobytes(), np.float32)[0])

_NC_CACHE = {}


def _emit_body(nc, tc, ap, pools, upto=99, accum_out=False):
    res, acc, big, dram = pools["res"], pools["acc"], pools["big"], pools["dram"]

    def rtile(shape, dtype, tag):
        return res.tile(shape, dtype, tag=tag, name=tag)

    # -------- resident loads: few big packed DMAs, first-use order --------
    MPK1 = rtile([128, 2, 2 * MPAD], F8, "mpk1")
    XQ2 = rtile([128, 2, 2 * QL], F8, "xq2")
    XAB = rtile([128, 2, 2 * N], F8, "xab")
    XKt = rtile([128, NJ, 2, 336], F8, "xk")
    WPK = rtile([128, 2, 3232], F8, "wpk")
    XF01 = rtile([128, 2 * QL], F32, "xf01")
    XF2 = rtile([64, QL], F32, "xf2")
    nc.sync.dma_start(out=MPK1, in_=ap["mpk1"])
    nc.sync.dma_start(out=XQ2, in_=ap["xq2"])
    # x pair tiles split in two chunks so attn1 scores can start early
    nc.sync.dma_start(out=XAB[:, :, 0:N], in_=ap["xab"][:, :, 0:N])
    nc.sync.dma_start(out=XKt[:, 0:NJ // 2], in_=ap["xk"][:, 0:NJ // 2])
    nc.sync.dma_start(out=XAB[:, :, N:2 * N], in_=ap["xab"][:, :, N:2 * N])
    nc.sync.dma_start(out=XKt[:, NJ // 2:NJ], in_=ap["xk"][:, NJ // 2:NJ])
    for t, nm in ((WPK, "wpk"), (XF01, "xf01"), (XF2, "xf2")):
        nc.sync.dma_start(out=t, in_=ap[nm])
    GNI = []
    for i, (off, sz) in enumerate(OTW):
        t = rtile([sz, 18], F32, f"gni_{i}")
        nc.sync.dma_start(out=t, in_=ap["gni"][off:off + sz, :])
        GNI.append(t)
    INDT = rtile([GROUPS, C], F32, "indt")
    nc.sync.dma_start(out=INDT, in_=ap["indt"])
    CTAB = rtile([128, 2, 2 * N], F8, "ctab")
    CKt = rtile([128, NJ, 2, 336], F8, "ck")
    GW01 = rtile([128, 2 * C8], BF16, "gw01")
    GW2t = rtile([65, C8], BF16, "gw2")
    DWP = rtile([128, 10, C], BF16, "dwp")
    DB16 = rtile([1, C], BF16, "db16")
    nc.sync.dma_start(out=CTAB[:, :, 0:N], in_=ap["ctab"][:, :, 0:N])
    nc.sync.dma_start(out=CKt[:, 0:NJ // 2], in_=ap["ck"][:, 0:NJ // 2])
    nc.sync.dma_start(out=CTAB[:, :, N:2 * N], in_=ap["ctab"][:, :, N:2 * N])
    nc.sync.dma_start(out=CKt[:, NJ // 2:NJ], in_=ap["ck"][:, NJ // 2:NJ])
    for t, nm in ((GW01, "gw01"), (GW2t, "gw2"), (DWP, "dwp"), (DB16, "db16")):
        nc.sync.dma_start(out=t, in_=ap[nm])

    MA1, MB1 = MPK1[:, :, 0:MPAD], MPK1[:, :, MPAD:2 * MPAD]
    XQA, XQB = XQ2[:, :, 0:QL], XQ2[:, :, QL:2 * QL]

    # x/context pair tiles packed as [A_lo | B_lo | A_hi | B_hi] (2048 cols
    # each) so each DMA chunk delivers matched A/B halves for 16 key tiles
    def kslicer(T):
        def ks(part, mt):
            off = (mt // 16) * 2 * (N // 2) + part * (N // 2) + (mt % 16) * 128
            return T[:, :, off:off + 128]
        return ks

    xsl = kslicer(XAB)
    csl = kslicer(CTAB)
    XK = [XKt[:, j] for j in range(NJ)]
    CK = [CKt[:, j] for j in range(NJ)]
    wo = [0, MPAD, 2 * MPAD, 2 * MPAD + C, 2 * MPAD + 2 * C, 2 * MPAD + 3 * C,
          2 * MPAD + 4 * C, 2 * MPAD + 5 * C, 2 * MPAD + 6 * C, 2 * MPAD + 7 * C]
    MA2, MB2 = WPK[:, :, wo[0]:wo[0] + MPAD], WPK[:, :, wo[1]:wo[1] + MPAD]
    WVA1, WVB1 = WPK[:, :, wo[2]:wo[2] + C], WPK[:, :, wo[3]:wo[3] + C]
    WPA1, WPB1 = WPK[:, :, wo[4]:wo[4] + C], WPK[:, :, wo[5]:wo[5] + C]
    WVA2, WVB2 = WPK[:, :, wo[6]:wo[6] + C], WPK[:, :, wo[7]:wo[7] + C]
    WPA2, WPB2 = WPK[:, :, wo[8]:wo[8] + C], WPK[:, :, wo[9]:wo[9] + C]
    XF = [XF01[:, 0:QL], XF01[:, QL:2 * QL], XF2]
    GB = [GNI[i][:, 0:2] for i in range(3)]
    INDsb = [GNI[i][:, 2:18] for i in range(3)]
    GW16 = [GW01[:, 0:C8], GW01[:, C8:2 * C8], GW2t]
    DW16 = [DWP[:, hh] for hh in range(10)]

    # ---------------- persistent on-chip state ----------------
    QM1A = rtile([128, 2, QL], F8, "qm1a")
    QM1B = rtile([128, 2, QL], F8, "qm1b")
    QM2A = rtile([128, 2, QL], F8, "qm2a")
    QM2B = rtile([128, 2, QL], F8, "qm2b")
    ES = [rtile([128, 2, QWIN], F8, f"es{j}") for j in range(NJ)]
    UA = [rtile([128, 2, QWIN], F8, f"ua{p}") for p in range(2)]
    UB = [rtile([128, 2, QWIN], F8, f"ub{p}") for p in range(2)]
    atA = [rtile([128, 2, QWIN], F8, f"ata{p}") for p in range(2)]
    atB = [rtile([128, 2, QWIN], F8, f"atb{p}") for p in range(2)]
    XNA = rtile([128, 2, QL], F8, "xna")
    XNB = rtile([128, 2, QL], F8, "xnb")
    X316 = [rtile([asz, QL], BF16, f"x316_{i}") for i, (_, asz) in enumerate(OT)]
    ONES16 = rtile([1, QWIN], BF16, "ones16")
    X2 = [rtile([sz, QL], F32, f"x2_{i}") for i, (_, sz) in enumerate(OTW)]
    X3 = [rtile([sz, QL], F32, f"x3_{i}") for i, (_, sz) in enumerate(OTW)]
    scratch = rtile([128, QL], F32, "scratch")

    # zero garbage-sensitive fp8 tiles (f32-bitcast memsets are 4x cheaper)
    for t in (QM1B, QM2B, UB[0], UB[1], atB[0], atB[1], XNB):
        nc.gpsimd.memset(t.bitcast(F32), 0.0)
    for p in range(2):
        nc.gpsimd.memset(atB[p][64:65, 0:1, :].bitcast(F32), F8_64_PAT)
    nc.gpsimd.memset(XNB[64:65, 0:1, :].bitcast(F32), F8_ONES_PAT)
    nc.gpsimd.memset(X316[2][64:65, :].bitcast(F32), BF16_ONES_PAT)
    nc.gpsimd.memset(ONES16.bitcast(F32), BF16_ONES_PAT)

    def mmslot(sz):
        return big.tile([128, QWIN], F32, tag="big", name="big")[0:sz, :]

    def qm_proj(MAt, MBt, SRCA, SRCB, DSTA, DSTB, qws=(0, 1)):
        """QM[c,q] = (1/32) * M''^T src'_q for the local query window."""
        for qw in qws:
            qsl = slice(qw * QWIN, (qw + 1) * QWIN)
            for ci, (off, sz) in enumerate(OT):
                pq = mmslot(sz)
                nc.tensor.matmul(pq, MAt[:, :, off:off + sz], SRCA[:, :, qsl],
                                 start=True, stop=False, perf_mode=DR)
                nc.tensor.matmul(pq, MBt[:, :, off:off + sz], SRCB[:, :, qsl],
                                 start=False, stop=True, perf_mode=DR)
                if ci < 2:
                    nc.vector.tensor_scalar_mul(out=DSTA[:, ci, qsl], in0=pq,
                                                scalar1=1.0 / WS)
                else:
                    nc.vector.tensor_scalar_mul(out=DSTB[0:65, 0, qsl], in0=pq,
                                                scalar1=1.0 / WS)

    def attention(ksl, XKT, QMA, QMB, WVA, WVB, WPA, WPB, resid_fn,
                  extra_work=()):
        """Software-pipelined: scores run one key-pair ahead of the U
        accumulation so PE never waits on exp; the qw0 tail (denominator,
        pv/pp projections) is woven between qw1's score pairs."""

        def emit_scores(qw, j):
            qsl = slice(qw * QWIN, (qw + 1) * QWIN)
            for h in range(2):
                mt = 2 * j + h
                psc = big.tile([128, QWIN], F32, tag="big", name="big")
                nc.tensor.matmul(psc, ksl(0, mt), QMA[:, :, qsl],
                                 start=True, stop=False, perf_mode=DR)
                nc.tensor.matmul(psc, ksl(1, mt), QMB[:, :, qsl],
                                 start=False, stop=True, perf_mode=DR)
                nc.scalar.activation(out=ES[j][:, h, :], in_=psc, func=AF.Exp,
                                     scale=SCALE / (MS / WS))

        def emit_U(u, XKT, j):
            for ci, (off, sz) in enumerate(OT):
                nc.tensor.matmul(u[ci], XKT[j][:, :, off:off + sz], ES[j],
                                 start=(j == 0), stop=(j == NJ - 1),
                                 perf_mode=DR)

        def tail_dve(u, par):
            # U -> fp8 at fixed 1/64 scale on the idle Act engine; the
            # softmax division happens later at the `at` stage.  In parallel,
            # reciprocal of the denominator row (already Sum(es)/64 via the
            # 1/64 ones-column) broadcasts to all partitions.
            rec = res.tile([1, QWIN], F32, tag="rec", name="rec", bufs=2)
            nc.vector.reciprocal(rec, u[2][64:65, :])
            dbc = res.tile([128, QWIN], F32, tag="dbc", name="dbc", bufs=2)
            nc.gpsimd.partition_broadcast(dbc, rec)
            nc.scalar.activation(out=UA[par][:, 0, :], in_=u[0], func=AF.Copy,
                                 scale=1.0 / US)
            nc.scalar.activation(out=UA[par][:, 1, :], in_=u[1], func=AF.Copy,
                                 scale=1.0 / US)
            nc.scalar.activation(out=UB[par][0:65, 0, :], in_=u[2],
                                 func=AF.Copy, scale=1.0 / US)
            return dbc

        def emit_pv(par, dbc):
            # pv = Wv'^T U_fp8; at = 2 * pv * (64/denom)  (64x attn-V out)
            pvs = [acc.tile([128, QWIN], F32, tag="u0", name="u0"),
                   acc.tile([128, QWIN], F32, tag="u1", name="u1"),
                   acc.tile([64, QWIN], F32, tag="py2", name="py2")]
            for ci, (off, sz) in enumerate(OTW):
                pv = pvs[ci]
                nc.tensor.matmul(pv, WVA[:, :, off:off + sz], UA[par],
                                 start=True, stop=False, perf_mode=DR)
                nc.tensor.matmul(pv, WVB[:, :, off:off + sz], UB[par],
                                 start=False, stop=True, perf_mode=DR)
                dst = atA[par][:, ci, :] if ci < 2 else atB[par][0:64, 0, :]
                nc.vector.scalar_tensor_tensor(
                    out=dst, in0=pv, scalar=2.0, in1=dbc[0:sz, :],
                    op0=AL.mult, op1=AL.mult)

        def emit_pp(par, qw):
            # pp = Wp'^T at (2048x attn out); resid folds the 1/2048
            for ci, (off, sz) in enumerate(OTW):
                pp = mmslot(sz)
                nc.tensor.matmul(pp, WPA[:, :, off:off + sz], atA[par],
                                 start=True, stop=False, perf_mode=DR)
                nc.tensor.matmul(pp, WPB[:, :, off:off + sz], atB[par],
                                 start=False, stop=True, perf_mode=DR)
                resid_fn(ci, qw, pp)

        def alloc_u():
            return [acc.tile([128, QWIN], F32, tag="u0", name="u0"),
                    acc.tile([128, QWIN], F32, tag="u1", name="u1"),
                    acc.tile([65, QWIN], F32, tag="u2", name="u2")]

        # qw0: scores two pairs ahead of U, scores first in each slot, so the
        # exp stream on Act runs back-to-back (it is the per-pair bottleneck)
        u0t = alloc_u()
        emit_scores(0, 0)
        emit_scores(0, 1)
        for j in range(2, NJ):
            emit_scores(0, j)
            emit_U(u0t, XKT, j - 2)
        emit_U(u0t, XKT, NJ - 2)
        emit_U(u0t, XKT, NJ - 1)
        # qw1: qw0's tail (denominator chain + pv/pp, which reuse the u0/u1
        # PSUM banks) woven between qw1's early score pairs; U starts after.
        emit_scores(1, 0)
        dbc0 = tail_dve(u0t, 0)
        emit_scores(1, 1)
        emit_pv(0, dbc0)
        emit_scores(1, 2)
        emit_pp(0, 0)
        emit_scores(1, 3)
        u1t = alloc_u()
        emit_U(u1t, XKT, 0)
        emit_U(u1t, XKT, 1)
        extra = list(extra_work)
        for j in range(4, NJ):
            emit_scores(1, j)
            emit_U(u1t, XKT, j - 2)
            if j % 4 == 1 and extra:
                extra.pop(0)()
        emit_U(u1t, XKT, NJ - 2)
        emit_U(u1t, XKT, NJ - 1)
        dbc1 = tail_dve(u1t, 1)
        emit_pv(1, dbc1)
        emit_pp(1, 1)
        for f in extra:
            f()

    # ======== attn1 (self-attention) ========
    qm_proj(MA1, MB1, XQA, XQB, QM1A, QM1B)
    if upto <= 1:
        return

    def resid1(ci, qw, pp):
        qsl = slice(qw * QWIN, (qw + 1) * QWIN)
        # x2 = 2*x + attn1   (XF holds 2*x)
        nc.vector.scalar_tensor_tensor(
            out=X2[ci][:, qsl], in0=pp, scalar=1.0 / (WS * US),
            in1=XF[ci][:, qsl], op0=AL.mult, op1=AL.add)

    attention(xsl, XK, QM1A, QM1B, WVA1, WVB1, WPA1, WPB1, resid1)
    if upto <= 2:
        return

    # ======== group-norm stats + AllReduce ========
    s12 = [rtile([sz, 2], F32, f"s12_{i}") for i, (_, sz) in enumerate(OTW)]
    for i, (_, sz) in enumerate(OTW):
        nc.vector.reduce_sum(out=s12[i][:, 0:1], in_=X2[i],
                             axis=mybir.AxisListType.X)
        nc.scalar.activation(
            out=scratch[0:sz, :], in_=X2[i], func=AF.Square,
            accum_out=s12[i][:, 1:2])
    pg = mmslot(GROUPS)[:, 0:2]
    for i in range(3):
        nc.tensor.matmul(pg, INDsb[i], s12[i], start=(i == 0), stop=(i == 2))
    g12 = rtile([GROUPS, 2], F32, "g12")
    nc.vector.tensor_copy(out=g12, in_=pg)
    ccin = dram.tile([GROUPS, 2], F32, tag="ccin", name="ccin")
    ccout = dram.tile([GROUPS, 2], F32, tag="ccout", name="ccout")
    nc.sync.dma_start(out=ccin, in_=g12)
    if not globals().get("_SKIP_COLLECTIVE"):
        nc.gpsimd.collective_compute(
            "AllReduce", AL.add,
            replica_groups=[[0, 1, 2, 3], [4, 5, 6, 7]],
            ins=[ccin.opt()], outs=[ccout.opt()])
    else:
        nc.sync.dma_start(out=ccout, in_=ccin)
    gg12 = rtile([GROUPS, 2], F32, "gg12")
    nc.sync.dma_start(out=gg12, in_=ccout)
    if upto <= 3:
        return

    # ======== finish group norm ========
    gtmp = rtile([GROUPS, 4], F32, "gtmp")
    grp2 = rtile([GROUPS, 2], F32, "grp2")
    inv = 1.0 / GCNT
    nc.vector.tensor_scalar_mul(out=grp2[:, 1:2], in0=gg12[:, 0:1], scalar1=inv)
    nc.vector.tensor_scalar_mul(out=gtmp[:, 0:1], in0=gg12[:, 1:2], scalar1=inv)
    nc.vector.tensor_mul(gtmp[:, 1:2], grp2[:, 1:2], grp2[:, 1:2])
    nc.vector.tensor_sub(gtmp[:, 2:3], gtmp[:, 0:1], gtmp[:, 1:2])
    epst = rtile([GROUPS, 1], F32, "epst")
    nc.vector.memset(epst, float(EPS))
    nc.scalar.activation(out=gtmp[:, 3:4], in_=gtmp[:, 2:3],
                         func=AF.Sqrt, bias=epst)
    nc.vector.reciprocal(grp2[:, 0:1], gtmp[:, 3:4])
    scss = []
    for i, (off, sz) in enumerate(OTW):
        pc = mmslot(sz)[:, 0:2]
        nc.tensor.matmul(pc, INDT[:, off:off + sz], grp2, start=True, stop=True)
        scs = rtile([sz, 4], F32, f"scs_{i}")
        nc.vector.tensor_mul(scs[:, 0:1], pc[:, 0:1], GB[i][:, 0:1])   # scale
        nc.vector.tensor_mul(scs[:, 3:4], pc[:, 1:2], scs[:, 0:1])
        nc.vector.tensor_sub(scs[:, 1:2], GB[i][:, 1:2], scs[:, 3:4])  # shift
        nc.vector.tensor_scalar_add(out=scs[:, 2:3], in0=scs[:, 0:1], scalar1=1.0)
        scss.append(scs)

    # ======== attn2 (cross-attention) ========
    # xn cast + QM2 projection per query window, so attn2 scores can start
    # before the second window's gnorm tail is done
    for qw in range(NQW):
        hsl = slice(qw * QWIN, (qw + 1) * QWIN)
        for i, (off, sz) in enumerate(OTW):
            dst = XNA[:, i, hsl] if i < 2 else XNB[0:64, 0, hsl]
            nc.vector.tensor_scalar(
                out=dst, in0=X2[i][:, hsl], scalar1=scss[i][:, 0:1],
                scalar2=scss[i][:, 1:2], op0=AL.mult, op1=AL.add)
        qm_proj(MA2, MB2, XNA, XNB, QM2A, QM2B, qws=(qw,))
    # x2 <- x2 + xn  (= x2*(1+scale) + shift), fp32, in place
    for i, (off, sz) in enumerate(OTW):
        nc.vector.tensor_scalar(
            out=X2[i], in0=X2[i], scalar1=scss[i][:, 2:3],
            scalar2=scss[i][:, 1:2], op0=AL.mult, op1=AL.add)

    def resid2(ci, qw, pp):
        qsl = slice(qw * QWIN, (qw + 1) * QWIN)
        nc.vector.scalar_tensor_tensor(
            out=X3[ci][:, qsl], in0=pp, scalar=1.0 / (WS * US),
            in1=X2[ci][:, qsl], op0=AL.mult, op1=AL.add)
        # bf16 copy for the FFN, per half, so geglu can start immediately
        sz = OTW[ci][1]
        nc.gpsimd.tensor_copy(out=X316[ci][0:sz, qsl], in_=X3[ci][:, qsl])

    # FFN machinery is defined before attention2 so the first geglu pairs of
    # window 0 can be woven into attn2's second score window.
    ffn_tts = {0: [None] * 5, 1: [None] * 5}

    def geglu(qw, j):
        qsl = slice(qw * QWIN, (qw + 1) * QWIN)
        tt = res.tile([128, 2 * QWIN], BF16, tag="tt", name="tt", bufs=3)
        for h in range(2):
            hh = 2 * j + h
            asl = slice(hh * 128, (hh + 1) * 128)
            gsl = slice(C4 + hh * 128, C4 + (hh + 1) * 128)
            pa = big.tile([128, QWIN], F32, tag="big", name="big")
            pg = big.tile([128, QWIN], F32, tag="big", name="big")
            for ci, (off, asz) in enumerate(OT):
                nc.tensor.matmul(pa, GW16[ci][:, asl], X316[ci][:, qsl],
                                 start=(ci == 0), stop=(ci == 2))
            for ci, (off, asz) in enumerate(OT):
                nc.tensor.matmul(pg, GW16[ci][:, gsl], X316[ci][:, qsl],
                                 start=(ci == 0), stop=(ci == 2))
            gg = res.tile([128, QWIN], BF16, tag="gg2", name="gg2", bufs=2)
            nc.scalar.activation(out=gg, in_=pg, func=AF.Silu, scale=1.702)
            nc.vector.scalar_tensor_tensor(
                out=tt[:, h * QWIN:(h + 1) * QWIN], in0=pa,
                scalar=1.0 / 1.702, in1=gg, op0=AL.mult, op1=AL.mult)
        ffn_tts[qw][j] = tt

    attention(csl, CK, QM2A, QM2B, WVA2, WVB2, WPA2, WPB2, resid2)
    if upto <= 4:
        return

    # ======== GEGLU FFN ========
    # YT reuses the X2 buffers (x2 is dead after resid2)
    YT = [rtile([sz, QL], F32, f"x2_{i}") for i, (_, sz) in enumerate(OTW)]
    for qw in range(NQW):
        qsl = slice(qw * QWIN, (qw + 1) * QWIN)
        py = [acc.tile([128, QWIN], F32, tag="u0", name="u0"),
              acc.tile([128, QWIN], F32, tag="u1", name="u1"),
              acc.tile([64, QWIN], F32, tag="py2", name="py2")]
        tts = ffn_tts[qw]

        def dense(j):
            for h in range(2):
                hh = 2 * j + h
                for ci, (off, sz) in enumerate(OTW):
                    nc.tensor.matmul(py[ci], DW16[hh][:, off:off + sz],
                                     tts[j][:, h * QWIN:(h + 1) * QWIN],
                                     start=(hh == 0), stop=False)

        for j in range(5):
            if tts[j] is None:
                geglu(qw, j)
            if j >= 1:
                dense(j - 1)
        dense(4)
        for ci, (off, sz) in enumerate(OTW):
            nc.tensor.matmul(py[ci], DB16[:, off:off + sz], ONES16,
                             start=False, stop=True)
            nc.vector.tensor_add(YT[ci][:, qsl], py[ci], X3[ci][:, qsl])
            if accum_out:
                nc.gpsimd.dma_start(out=ap["yt"][off:off + sz, qsl],
                                    in_=YT[ci][:, qsl], accum_op=AL.add)
            else:
                nc.sync.dma_start(out=ap["yt"][off:off + sz, qsl],
                                  in_=YT[ci][:, qsl])

    if "tick" in ap:
        tick = rtile([1, 4], F32, "tick")
        for i in range(3):
            nc.vector.tensor_copy(out=tick[0:1, i:i + 1],
                                  in_=YT[i][0:1, QL - 1:QL])
        nc.sync.dma_start(out=ap["tick"], in_=tick)


_SHAPES = {
    "mpk1": ([128, 2, 2 * MPAD], F8),
    "xq2": ([128, 2, 2 * QL], F8),
    "xab": ([128, 2, 2 * N], F8),
    "xk": ([128, NJ, 2, 336], F8),
    "wpk": ([128, 2, 3232], F8),
    "xf01": ([128, 2 * QL], F32), "xf2": ([64, QL], F32),
    "gni": ([C, 18], F32), "indt": ([GROUPS, C], F32),
    "ctab": ([128, 2, 2 * N], F8),
    "ck": ([128, NJ, 2, 336], F8),
    "gw01": ([128, 2 * C8], BF16), "gw2": ([65, C8], BF16),
    "dwp": ([128, 10, C], BF16), "db16": ([1, C], BF16),
}


def _declare(nc, tick=False):
    ap = {}
    for name, (shape, dt) in _SHAPES.items():
        ap[name] = nc.dram_tensor(name, shape, dt, kind="ExternalInput").ap()
    ap["yt"] = nc.dram_tensor("yt", [C, QL], F32, kind="ExternalOutput").ap()
    if tick:
        ap["tick"] = nc.dram_tensor("tick", [1, 4], F32, kind="ExternalOutput").ap()
    return ap


def _emit_all(nc, ap, rep, upto, accum_out):
    with tile.TileContext(nc) as tc:
        with (
            tc.tile_pool(name="res", bufs=1) as res,
            tc.tile_pool(name="acc", bufs=1, space="PSUM") as acc,
            tc.tile_pool(name="big", bufs=4, space="PSUM") as big,
            tc.tile_pool(name="dram", bufs=1, space="DRAM") as dram,
        ):
            pools = {"res": res, "acc": acc, "big": big, "dram": dram}
            for _ in range(rep):
                _emit_body(nc, tc, ap, pools, upto=upto, accum_out=accum_out)
    nc.finalize()


def _build(rep=1, accum_out=False, tick=False):
    key = (rep, accum_out, tick)
    if key in _NC_CACHE:
        return _NC_CACHE[key]
    nc = bacc.Bacc("TRN2", target_bir_lowering=False, debug=False,
                   num_devices=NCORES)
    ap = _declare(nc, tick=tick)
    _emit_all(nc, ap, rep, 99, accum_out)
    _NC_CACHE[key] = nc
    return nc


def _build_single(rep=1, upto=99):
    """Single-core, collective-free variant for TimelineSim analysis."""
    nc = bacc.Bacc("TRN2", target_bir_lowering=False, debug=False,
                   num_devices=1)
    ap = _declare(nc, tick=False)
    globals()["_SKIP_COLLECTIVE"] = True
    try:
        _emit_all(nc, ap, rep, upto, False)
    finally:
        globals()["_SKIP_COLLECTIVE"] = False
    return nc


def _pack_pairs(m, width=None):
    """[rows<=322, cols] fp32 -> (A, B) [128, 2, width] fp8.
    A: rows 0..255; B subtile0: rows 256.., subtile1: zeros."""
    rows, cols = m.shape
    width = width or cols
    A = np.zeros((128, 2, width), np.float32)
    Bt = np.zeros((128, 2, width), np.float32)
    A[:, 0, :cols] = m[0:128]
    A[:, 1, :cols] = m[128:256]
    nb = rows - 256
    Bt[0:nb, 0, :cols] = m[256:rows]
    return _f8(A), _f8(Bt)


def _f8(a):
    return np.ascontiguousarray(np.clip(a, -240, 240)).astype(f8)


def _prep_inputs(inputs):
    """Host-side packing/sharding. Returns in_maps for the 8 cores."""
    f32 = np.float32
    g = {k: np.asarray(v, f32) for k, v in inputs.items()}
    x = g["x"].reshape(B, N, C)
    ctx = g["context"].reshape(B, N, C)

    def fold_m(wq, bq, wk, bk):
        m = np.zeros((321, 321), np.float64)
        m[:320, :320] = wq.astype(np.float64) @ wk.astype(np.float64).T
        m[320, :320] = bq.astype(np.float64) @ wk.astype(np.float64).T
        m[:320, 320] = wq.astype(np.float64) @ bk.astype(np.float64)
        m[320, 320] = float(bq @ bk)
        return (m * MS).astype(f32)

    def aug_w(w, b, scale=WS, bscale=None):
        bs = scale if bscale is None else bscale
        return np.concatenate([w * scale, (b * bs).reshape(1, -1)], axis=0)

    m1a, m1b = _pack_pairs(fold_m(g["sa_q_w"], g["sa_q_b"], g["sa_k_w"], g["sa_k_b"]), MPAD)
    m2a, m2b = _pack_pairs(fold_m(g["ca_q_w"], g["ca_q_b"], g["ca_k_w"], g["ca_k_b"]), MPAD)
    wv1a, wv1b = _pack_pairs(aug_w(g["sa_v_w"], g["sa_v_b"], bscale=WS * US))
    wp1a, wp1b = _pack_pairs(aug_w(g["sa_p_w"], g["sa_p_b"]))
    wv2a, wv2b = _pack_pairs(aug_w(g["ca_v_w"], g["ca_v_b"], bscale=WS * US))
    wp2a, wp2b = _pack_pairs(aug_w(g["ca_p_w"], g["ca_p_b"]))
    mpk1 = np.concatenate([m1a, m1b], axis=2)
    wpk = np.concatenate([m2a, m2b, wv1a, wv1b, wp1a, wp1b,
                          wv2a, wv2b, wp2a, wp2b], axis=2)
    bf = ml_dtypes.bfloat16
    gwf = np.concatenate([g["geglu_w"], g["geglu_b"].reshape(1, -1)],
                         axis=0).astype(bf)                     # [321, C8]
    gw01 = np.ascontiguousarray(
        np.concatenate([gwf[0:128], gwf[128:256]], axis=1))     # [128, 2*C8]
    gw2 = np.ascontiguousarray(gwf[256:321])                    # [65, C8]
    dwp = np.ascontiguousarray(
        g["dense_w"].astype(bf).reshape(10, 128, C).transpose(1, 0, 2))
    db16 = g["dense_b"].reshape(1, C).astype(bf)

    gb = np.stack([g["ca_norm_g"], g["ca_norm_b"]], axis=1)
    ind = np.zeros((C, GROUPS), f32)
    ind[np.arange(C), np.arange(C) // GSIZE] = 1.0
    indt = np.ascontiguousarray(ind.T)
    gni = np.ascontiguousarray(np.concatenate([gb, ind], axis=1))  # [C, 18]

    def chan_pairs(xb):
        """x [N, C] -> A/B pair tiles [128, 2, N] (channels on partitions)."""
        xt = np.concatenate([xb.T, np.ones((1, N), f32)], axis=0)  # [321, N]
        return _pack_pairs(xt)

    def key_tiles(xb):
        """x [N, C] -> [128, NJ, 2, 336] (keys on partitions, +ones col)."""
        out = np.zeros((128, NJ, 2, 336), f32)
        xr = xb.reshape(NJ, 2, 128, C)  # [j, t, p, c]
        out[:, :, :, 0:C] = xr.transpose(2, 0, 1, 3)
        out[:, :, :, C] = 1.0 / US  # pre-scales the denominator row by 1/64
        return _f8(out)

    xa = [None] * B
    xb_ = [None] * B
    cta = [None] * B
    ctb = [None] * B
    xk = [None] * B
    ck = [None] * B
    for b in range(B):
        xa[b], xb_[b] = chan_pairs(x[b])
        cta[b], ctb[b] = chan_pairs(ctx[b])
        xk[b] = key_tiles(x[b])
        ck[b] = key_tiles(ctx[b])

    shared = {
        "mpk1": mpk1, "wpk": wpk,
        "gw01": gw01, "gw2": gw2, "dwp": dwp, "db16": db16,
        "gni": gni, "indt": indt,
    }
    # [A_lo | B_lo | A_hi | B_hi] chunking (2048 cols each half)
    hN = N // 2
    xab = [np.concatenate([xa[b][:, :, 0:hN], xb_[b][:, :, 0:hN],
                           xa[b][:, :, hN:N], xb_[b][:, :, hN:N]], axis=2)
           for b in range(B)]
    ctab = [np.concatenate([cta[b][:, :, 0:hN], ctb[b][:, :, 0:hN],
                            cta[b][:, :, hN:N], ctb[b][:, :, hN:N]], axis=2)
            for b in range(B)]
    in_maps = []
    for c in range(NCORES):
        b = c // 4
        q0 = (c % 4) * QL
        xf = 2.0 * x[b, q0:q0 + QL, :].T  # [C, QL]
        m = {
            "xab": xab[b], "ctab": ctab[b],
            "xq2": np.ascontiguousarray(
                np.concatenate([xa[b][:, :, q0:q0 + QL],
                                xb_[b][:, :, q0:q0 + QL]], axis=2)),
            "xk": xk[b], "ck": ck[b],
            "xf01": np.ascontiguousarray(
                np.concatenate([xf[0:128], xf[128:256]], axis=1)),
            "xf2": np.ascontiguousarray(xf[256:320]),
        }
        m.update(shared)
        in_maps.append(m)
    return in_maps


def kernel(**inputs):
    in_maps = _prep_inputs(inputs)
    nc = _build()
    res = run_bass_kernel_spmd(nc, in_maps, list(range(NCORES)))
    out = np.zeros((B, N, C), np.float32)
    for c in range(NCORES):
        b = c // 4
        q0 = (c % 4) * QL
        out[b, q0:q0 + QL, :] = res.results[c]["yt"].T
    return out.reshape(B, H, W, C)
